# revision 2
# baseline (speedup 1.0000x reference)
"""Trainium2 Bass kernel for nn_LongTermFeatureBank (8-core SPMD, batch-sharded)."""
import os
import numpy as np
import ml_dtypes

import concourse.bass as bass
import concourse.tile as tile
from concourse import bacc, mybir
from concourse.bass_utils import run_bass_kernel_spmd

F32 = mybir.dt.float32
BF16 = mybir.dt.bfloat16
X = mybir.AxisListType.X
ADD = mybir.AluOpType.add
SUB = mybir.AluOpType.subtract
MUL = mybir.AluOpType.mult
BYP = mybir.AluOpType.bypass
EXP = mybir.ActivationFunctionType.Exp
RELU = mybir.ActivationFunctionType.Relu
SQRT = mybir.ActivationFunctionType.Sqrt

B, D1, D2, HW = 128, 512, 16, 49
NC = 8
BL = B // NC          # 16 local batch
R = BL * D2           # 256 rows (b,d2)
ISD = float(1.0 / np.sqrt(D1))
NB = 4                # b-granularity of x sub-chunk DMAs
LAST = {}


def _declare(nc):
    ap = {}
    def di(name, shape, dt=F32):
        ap[name] = nc.dram_tensor(name, list(shape), dt, kind="ExternalInput").ap()
    di("x_s", (BL, D1, D2, 7, 7))
    for s in range(2):
        di(f"waT{s}", (D1, D1)); di(f"wbT{s}", (D1, D1)); di(f"wfT{s}", (D1, D1))
        di(f"ba{s}", (1, D1)); di(f"bb{s}", (1, D1)); di(f"bf{s}", (1, D1))
    di("lngT", (D1, D2)); di("lnbT", (D1, D2))
    di("w1T", (2 * D2 * D1, 200), BF16); di("b1", (1, 200))
    di("w2T", (200, 50)); di("b2T", (50, 1))
    di("w3T", (50, 10)); di("b3T", (10, 1))
    di("w4T", (10, 128)); di("b4T", (128, 1))
    di("p8", (128, 16)); di("p8t", (16, 128)); di("maskbd", (128, 128))
    di("idn", (128, 128)); di("ones", (1, 256)); di("ones128", (128, 1))
    ap["outT"] = nc.dram_tensor("outT", [128, BL], F32, kind="ExternalOutput").ap()
    return ap


def _build(nc, tc, ap, collective=True, ablate=0):
    MM = nc.tensor.matmul
    cp = nc.vector.tensor_copy
    tt = nc.vector.tensor_tensor
    from contextlib import ExitStack
    ctx = ExitStack()
    P = ctx.enter_context(tc.tile_pool(name="persist", bufs=1))
    PX = ctx.enter_context(tc.tile_pool(name="xstage", bufs=3))
    PP = ctx.enter_context(tc.tile_pool(name="ps", bufs=6, space="PSUM"))
    PH = ctx.enter_context(tc.tile_pool(name="ph", bufs=1, space="PSUM"))
    PD = ctx.enter_context(tc.tile_pool(name="dram", bufs=4, space="DRAM"))

    def ps(tag="ps"):
        return PP.tile([128, 512], F32, tag=tag, name=tag)

    # ---- persistent SBUF loads ----
    def ld(name, shape, src, tag=None, eng=None):
        t = P.tile(list(shape), src.dtype if hasattr(src, "dtype") else F32,
                   tag=tag or name)
        (eng or nc.sync).dma_start(t[:], src)
        return t
    w = {}
    for s in range(2):
        for nm in ("waT", "wbT", "wfT"):
            w[f"{nm}{s}"] = ld(f"{nm}{s}", (128, 4, D1),
                               ap[f"{nm}{s}"].rearrange("(c p) o -> p c o", p=128),
                               eng=nc.gpsimd)
        for nm in ("ba", "bb", "bf"):
            w[f"{nm}{s}"] = ld(f"{nm}{s}", (1, D1), ap[f"{nm}{s}"])
    lngT = ld("lngT", (128, 4, D2), ap["lngT"].rearrange("(c p) k -> p c k", p=128))
    lnbT = ld("lnbT", (128, 4, D2), ap["lnbT"].rearrange("(c p) k -> p c k", p=128))
    ones = ld("ones", (1, 256), ap["ones"])
    ones128 = ld("ones128", (128, 1), ap["ones128"])
    p8 = ld("p8", (128, 16), ap["p8"])
    p8t = ld("p8t", (16, 128), ap["p8t"])
    maskbd = ld("maskbd", (128, 128), ap["maskbd"])
    idn = ld("idn", (128, 128), ap["idn"])
    b1 = ld("b1", (1, 200), ap["b1"])
    w2a = ld("w2a", (128, 50), ap["w2T"][0:128, :])
    w2b = ld("w2b", (72, 50), ap["w2T"][128:200, :])
    w3 = ld("w3", (50, 10), ap["w3T"]); w4 = ld("w4", (10, 128), ap["w4T"])
    b2T = ld("b2T", (50, 1), ap["b2T"]); b3T = ld("b3T", (10, 1), ap["b3T"])
    b4T = ld("b4T", (128, 1), ap["b4T"])

    # ---- pooling: x -> xpT[c] [128 d1, 256 (b,d2)] ----
    xr = ap["x_s"].rearrange("b (c p) d h w -> c p b (d h w)", p=128)
    xpT = [P.tile([128, R], F32, tag=f"xpT{c}", name=f"xpT{c}") for c in range(4)]
    for c in range(4):
        for b0 in range(0, BL, NB):
            xt = PX.tile([128, NB, D2 * HW], F32, tag="xt", name="xt")
            nc.sync.dma_start(xt[:], xr[c, :, b0:b0 + NB, :])
            nc.vector.reduce_max(
                out=xpT[c][:, b0 * D2:(b0 + NB) * D2],
                in_=xt.rearrange("p n (k h) -> p (n k) h", h=HW), axis=X)

    # w1T load placed after x DMAs in program order (overlaps stack compute)
    w1 = ld("w1T", (128, 128, 200), ap["w1T"].rearrange("(c p) o -> p c o", p=128), eng=nc.scalar)

    def stack(s, prevT):
        wa, wb, wf = w[f"waT{s}"], w[f"wbT{s}"], w[f"wfT{s}"]
        ba, bb, bf = w[f"ba{s}"], w[f"bb{s}"], w[f"bf{s}"]
        # a_rows [2][128 (b,d2), 512 t]
        a_rows = []
        for h in range(2):
            pa = ps()
            for k in range(4):
                MM(pa[:, :D1], prevT[k][:, h * 128:(h + 1) * 128], wa[:, k, :],
                   start=(k == 0), stop=False)
            MM(pa[:, :D1], ones[0:1, 0:128], ba[:], start=False, stop=True)
            t = P.tile([128, D1], F32, tag=f"ar{h}", name=f"ar{h}")
            cp(out=t[:], in_=pa[:, :D1]); a_rows.append(t)
        # aT / bprojT [4][128 o, 256 rows]
        def proj(wm, bv, src, tag):
            out = []
            for oc in range(4):
                pa = ps()
                for k in range(4):
                    MM(pa[:, :R], wm[:, k, oc * 128:(oc + 1) * 128], src[k][:],
                       start=(k == 0), stop=False)
                MM(pa[:, :R], bv[0:1, oc * 128:(oc + 1) * 128], ones[0:1, :R],
                   start=False, stop=True)
                t = P.tile([128, R], F32, tag=f"{tag}{oc}", name=f"{tag}{oc}")
                cp(out=t[:], in_=pa[:, :R]); out.append(t)
            return out
        aT = proj(wa, ba, prevT, "aT")
        bT = proj(wb, bb, xpT, "bT")
        # cross scores + exp + mask
        em = []
        for h in range(2):
            pc = ps()
            for k in range(4):
                MM(pc[:, :128], bT[k][:, h * 128:(h + 1) * 128],
                   aT[k][:, h * 128:(h + 1) * 128], start=(k == 0), stop=(k == 3))
            e = P.tile([128, 128], F32, tag=f"em{h}", name=f"em{h}")
            nc.scalar.activation(out=e[:], in_=pc[:, :128], func=EXP, scale=ISD)
            tt(e[:], e[:], maskbd[:], MUL)
            em.append(e)
        # compact E [16 d, 256 (b,c)]
        E = P.tile([16, 256], F32, tag="E", name="E")
        for h in range(2):
            pe = ps()
            MM(pe[:16, :128], p8[:], em[h][:], start=True, stop=True)
            cp(out=E[:, h * 128:(h + 1) * 128], in_=pe[:16, :128])
        # local softmax sum over b -> [16 d, 16 c]
        Sl = P.tile([16, 16], F32, tag="Sl", name="Sl")
        nc.vector.reduce_sum(out=Sl[:], in_=E.rearrange("d (b c) -> d c b", c=16),
                             axis=X)
        # AllGather of local sums
        ib = PD.tile([16, 16], F32, tag="agin", name="agin")
        ob = PD.tile([128, 16], F32, tag="agout", name="agout")
        nc.sync.dma_start(ib[:], Sl[:])
        if collective:
            nc.gpsimd.collective_compute(
                "AllGather", BYP, ins=[ib.opt()], outs=[ob.opt()],
                replica_groups=[list(range(NC))])
        g = P.tile([16, 8, 16], F32, tag="gsum", name="gsum")
        if collective:
            nc.sync.dma_start(g[:], ob.rearrange("(r d) c -> d r c", r=8))
        else:
            for r_ in range(8):
                nc.sync.dma_start(g[:, r_, :], ib[:])
        Sg = P.tile([16, 16], F32, tag="Sg", name="Sg")
        nc.vector.reduce_sum(out=Sg[:], in_=g.rearrange("d r c -> d c r"), axis=X)
        rS = P.tile([16, 16], F32, tag="rS", name="rS")
        nc.vector.reciprocal(out=rS[:], in_=Sg[:])
        AB = P.tile([16, 256], F32, tag="AB", name="AB")
        tt(AB.rearrange("d (b c) -> d c b", c=16),
           E.rearrange("d (b c) -> d c b", c=16),
           rS[:, :, None].to_broadcast((16, 16, 16)), MUL)
        # block-diag ab, then ctxT
        bd = []
        for h in range(2):
            pb = ps()
            MM(pb[:, :128], p8t[:], AB[:, h * 128:(h + 1) * 128],
               start=True, stop=True)
            t = P.tile([128, 128], F32, tag=f"bd{h}", name=f"bd{h}")
            tt(t[:], pb[:, :128], maskbd[:], MUL)
            bd.append(t)
        ctxT, sq = [], []
        for k in range(4):
            pc = ps()
            for h in range(2):
                MM(pc[:, h * 128:(h + 1) * 128],
                   a_rows[h][:, k * 128:(k + 1) * 128], bd[h][:],
                   start=True, stop=True, skip_group_check=True)
            t = P.tile([128, R], F32, tag=f"ctxT{k}", name=f"ctxT{k}")
            cp(out=t[:], in_=pc[:, :R]); ctxT.append(t)
            q = P.tile([128, R], F32, tag=f"sq{k}", name=f"sq{k}")
            tt(q[:], t[:], t[:], MUL); sq.append(q)
        # LN stats via ones-matmuls
        ps1, ps2 = ps(), ps()
        for k in range(4):
            MM(ps1[:1, :R], ones128[:], ctxT[k][:], start=(k == 0), stop=(k == 3))
            MM(ps2[:1, :R], ones128[:], sq[k][:], start=(k == 0), stop=(k == 3))
        st = P.tile([1, 34], F32, tag="st", name="st")  # 0:16 mean 16:32 rstd 32 tmp 33 eps
        nc.vector.memset(st[:, 33:34], 1e-5)
        nc.vector.reduce_sum(out=st[:, 0:16],
                             in_=ps1[:1, :R].rearrange("o (b c) -> o b c", c=16),
                             axis=X)
        nc.vector.reduce_sum(out=st[:, 16:32],
                             in_=ps2[:1, :R].rearrange("o (b c) -> o b c", c=16),
                             axis=X)
        nc.vector.tensor_scalar_mul(st[:, 0:16], st[:, 0:16], 1.0 / 8192.0)
        nc.vector.tensor_scalar_mul(st[:, 16:32], st[:, 16:32], 1.0 / 8192.0)
        # var = E[x^2] - mean^2 (elementwise over the 16 b)
        tmp = P.tile([1, 16], F32, tag="lntmp", name="lntmp")
        tt(tmp[:], st[:, 0:16], st[:, 0:16], MUL)
        tt(st[:, 16:32], st[:, 16:32], tmp[:], SUB)
        nc.scalar.activation(out=st[:, 16:32], in_=st[:, 16:32], func=SQRT,
                             bias=st[:, 33:34])
        nc.vector.reciprocal(out=st[:, 16:32], in_=st[:, 16:32])
        pbst = P.tile([128, 32], F32, tag="pbst", name="pbst")
        nc.gpsimd.partition_broadcast(pbst[:], st[:, 0:32])
        # normalize + affine + relu -> eT
        eT = []
        for k in range(4):
            t = P.tile([128, R], F32, tag=f"eT{k}", name=f"eT{k}")
            v3 = t.rearrange("p (b c) -> p b c", c=16)
            tt(v3, ctxT[k].rearrange("p (b c) -> p b c", c=16),
               pbst[:, 0:16, None].to_broadcast((128, 16, 16)), SUB)
            tt(v3, v3, pbst[:, 16:32, None].to_broadcast((128, 16, 16)), MUL)
            vcb = t.rearrange("p (b c) -> p c b", c=16)
            tt(vcb, vcb, lngT[:, k, :, None].to_broadcast((128, 16, 16)), MUL)
            tt(vcb, vcb, lnbT[:, k, :, None].to_broadcast((128, 16, 16)), ADD)
            nc.scalar.activation(out=t[:], in_=t[:], func=RELU)
            eT.append(t)
        # fc1 + residual -> new_xT
        nxT = []
        for oc in range(4):
            pn = ps()
            for k in range(4):
                MM(pn[:, :R], wf[:, k, oc * 128:(oc + 1) * 128], eT[k][:],
                   start=(k == 0), stop=False)
            MM(pn[:, :R], bf[0:1, oc * 128:(oc + 1) * 128], ones[0:1, :R],
               start=False, stop=True)
            t = P.tile([128, R], F32, tag=f"nxT{s}_{oc}", name=f"nxT{s}_{oc}")
            tt(t[:], pn[:, :R], prevT[oc][:], ADD)
            nxT.append(t)
        return nxT

    # classifier phase A: xp-half of feat (overlaps x-DMA / stacks)
    fx = []
    for k in range(4):
        t = P.tile([128, R], BF16, tag=f"fx{k}", name=f"fx{k}")
        cp(out=t[:], in_=xpT[k][:]); fx.append(t)
    ph1 = PH.tile([128, 512], F32, tag="ph1", name="ph1")
    def cls_half(which, fx4, jbase):
        j = jbase
        for d2 in range(D2):
            for c4 in range(4):
                grp = j % 4
                lhsT = fx4[c4].rearrange("p (b c) -> p c b", c=16)[:, d2, :]
                MM(ph1[32 * grp:32 * grp + 16, :200], lhsT, w1[:, j, :],
                   start=(j < 4), stop=(j >= 125), tile_position=(0, 32 * grp),
                   skip_group_check=True)
                j += 1
    cls_half(0, fx, 0)
    if ablate >= 2:
        nxT = xpT
    else:
        nxT = stack(0, xpT)
        nxT = stack(1, nxT)
    if ablate >= 1:
        t0 = P.tile([128, BL], F32, tag="oTa", name="oTa")
        cp(out=t0[:], in_=nxT[0][:, :BL])
        nc.sync.dma_start(ap["outT"], t0[:])
        ctx.close()
        return

    # ---- classifier ----
    fx = []
    for src in (xpT, nxT):
        for k in range(4):
            t = P.tile([128, R], BF16, tag=f"fx{len(fx)}")
            cp(out=t[:], in_=src[k][:]); fx.append(t)
    ph1 = ps()
    j = 0
    for which in range(2):
        for d2 in range(D2):
            for c4 in range(4):
                grp = j % 4
                lhsT = fx[which * 4 + c4].rearrange("p (b c) -> p c b", c=16)[:, d2, :]
                MM(ph1[32 * grp:32 * grp + 16, :200], lhsT, w1[:, j, :],
                   start=(j < 4), stop=(j >= 125), tile_position=(0, 32 * grp),
                   skip_group_check=True)
                j += 1
    MM(ph1[0:16, :200], ones[0:1, 0:16], b1[:], start=False, stop=True,
       tile_position=(0, 0), skip_group_check=True)
    h1 = P.tile([16, 200], F32, tag="h1", name="h1")
    cp(out=h1[:], in_=ph1[0:16, :200])
    tt(h1[:], h1[:], ph1[32:48, :200], ADD)
    tt(h1[:], h1[:], ph1[64:80, :200], ADD)
    tt(h1[:], h1[:], ph1[96:112, :200], ADD)
    nc.scalar.activation(out=h1[:], in_=h1[:], func=RELU)
    # transpose h1 -> h1T (two pieces)
    pt = ps()
    nc.tensor.transpose(pt[:, :16], h1[:, 0:128], idn[:16, :16])
    h1a = P.tile([128, 16], F32, tag="h1a", name="h1a"); cp(out=h1a[:], in_=pt[:, :16])
    pt2 = ps()
    nc.tensor.transpose(pt2[:72, :16], h1[:, 128:200], idn[:16, :16])
    h1b = P.tile([72, 16], F32, tag="h1b", name="h1b"); cp(out=h1b[:], in_=pt2[:72, :16])
    # fc2/fc3/fc4
    p2 = ps()
    MM(p2[:50, :BL], w2a[:], h1a[:], start=True, stop=False)
    MM(p2[:50, :BL], w2b[:], h1b[:], start=False, stop=True)
    h2 = P.tile([50, BL], F32, tag="h2", name="h2")
    nc.scalar.activation(out=h2[:], in_=p2[:50, :BL], func=RELU, bias=b2T[:])
    p3 = ps()
    MM(p3[:10, :BL], w3[:], h2[:], start=True, stop=True)
    h3 = P.tile([10, BL], F32, tag="h3", name="h3")
    nc.scalar.activation(out=h3[:], in_=p3[:10, :BL], func=RELU, bias=b3T[:])
    p4 = ps()
    MM(p4[:, :BL], w4[:], h3[:], start=True, stop=True)
    oT = P.tile([128, BL], F32, tag="oT", name="oT")
    tt(oT[:], p4[:, :BL], b4T[:, 0, None].to_broadcast((128, BL)), ADD)
    nc.sync.dma_start(ap["outT"], oT[:])
    ctx.close()


def build_program(collective=True, ablate=0):
    nc = bacc.Bacc("TRN2", target_bir_lowering=False, debug=False,
                   num_devices=NC if collective else 1)
    ap = _declare(nc)
    with tile.TileContext(nc) as tc:
        _build(nc, tc, ap, collective=collective, ablate=ablate)
    nc.compile()
    return nc


def make_in_maps(inputs):
    x = np.asarray(inputs["x"], np.float32)
    aw = np.asarray(inputs["attn_w"], np.float32)
    ab = np.asarray(inputs["attn_b"], np.float32)
    I16 = np.eye(16, dtype=np.float32)
    shared = {
        "lngT": np.ascontiguousarray(np.asarray(inputs["ln_g"], np.float32).T),
        "lnbT": np.ascontiguousarray(np.asarray(inputs["ln_b"], np.float32).T),
        "w1T": np.ascontiguousarray(
            np.asarray(inputs["w1"], np.float32).T).astype(ml_dtypes.bfloat16),
        "b1": np.asarray(inputs["b1"], np.float32).reshape(1, 200),
        "w2T": np.ascontiguousarray(np.asarray(inputs["w2"], np.float32).T),
        "b2T": np.asarray(inputs["b2"], np.float32).reshape(50, 1),
        "w3T": np.ascontiguousarray(np.asarray(inputs["w3"], np.float32).T),
        "b3T": np.asarray(inputs["b3"], np.float32).reshape(10, 1),
        "w4T": np.ascontiguousarray(np.asarray(inputs["w4"], np.float32).T),
        "b4T": np.asarray(inputs["b4"], np.float32).reshape(128, 1),
        "p8": np.tile(I16, (8, 1)), "p8t": np.tile(I16, (1, 8)),
        "maskbd": np.kron(np.eye(8, dtype=np.float32), np.ones((16, 16), np.float32)),
        "idn": np.eye(128, dtype=np.float32),
        "ones": np.ones((1, 256), np.float32),
        "ones128": np.ones((128, 1), np.float32),
    }
    for s in range(2):
        shared[f"waT{s}"] = np.ascontiguousarray(aw[s, 0].T)
        shared[f"wbT{s}"] = np.ascontiguousarray(aw[s, 1].T)
        shared[f"wfT{s}"] = np.ascontiguousarray(aw[s, 3].T)
        shared[f"ba{s}"] = ab[s, 0].reshape(1, D1)
        shared[f"bb{s}"] = ab[s, 1].reshape(1, D1)
        shared[f"bf{s}"] = ab[s, 3].reshape(1, D1)
    return [{**shared, "x_s": np.ascontiguousarray(x[r * BL:(r + 1) * BL])}
            for r in range(NC)]


def kernel(**inputs):
    nc = build_program()
    in_maps = make_in_maps(inputs)
    res = run_bass_kernel_spmd(
        nc, in_maps, core_ids=list(range(NC)),
        trace=bool(os.environ.get("KTRACE")),
        tmpdir=os.environ.get("KTRACE_DIR"))
    LAST["results"] = res
    outs = [res.results[r]["outT"] for r in range(NC)]
    return np.concatenate(outs, axis=1).T.astype(np.float32)

